# revision 4
# baseline (speedup 1.0000x reference)
"""Masked dot-product attention on 8 Trainium2 NeuronCores — v2.

Problem: B=2, H=16, S=2048, D=64 fp32; scores = QK^T/sqrt(1024),
key-mask [B,S] with -1e9 on masked keys, softmax over keys, out = W @ V.

v2 strategy (data-parallel over the 32 (b,h) pairs, 4 per core):
 - K/V host-compacted to kept keys (masked keys get exactly-zero weight),
   zero-padded to a multiple of 128. Pad rows have all-zero V|ones rows so
   they are annihilated by the PV matmul.
 - QK runs as ONE fp8 DoubleRow matmul per (k-tile, q-quarter) with
   contraction 128 = [Q8 | R8] x [K8 | K8]: slice 0 carries e4m3(Q), slice 1
   the Q-quantization residual e4m3(Q - Q8) against the same K8. Cost is
   0.5 cycles/row (2x over f32r); Q-side quantization error cancels
   in-matmul, only K8's ~3.6% per-element quantization remains (output rel
   err ~9e-3 vs the 2e-2 gate).
 - exp is split between engines at k-subtile granularity: ACT computes
   exact bf16 exp on each group's first subtiles; DVE approximates the
   group's last SCHN-pattern subtiles with a two-point averaged Schraudolph
   (max rel err ~1.2%, mostly cancelled by softmax normalization):
   I1 = i16(s*C1 + C2) (f32->i16 tensor_scalar), I2 = I1 + GAP (i16, 4x
   DVE mode), E = bf16(I1) + bf16(I2) (tensor_tensor add, 2x mode; the
   second point's weight is folded into GAP as an exponent shift).
 - PV: lhsT = V1 = [V | ones] bf16 [128k, 65]; PSUM acc [65, 512]
   accumulates numerator and denominator together.
 - Epilogue per (pair, quarter): DVE copies acc -> SBUF; 4 PE transposes
   (f32) bring q onto partitions as pt [128, 4, 65]; one batched
   DVE reciprocal [128, 4]; one DVE scalar_tensor_tensor multiplies all
   4x64 output columns by the per-(partition, j) reciprocal broadcast along
   d (free-dim 0-stride AP).
 - PSUM: scores [128, GROUP=2, 512] x3 bufs (6 banks) + acc (1) + pt (1)
   = 8 banks; 3 score buffers break the QK->exp->buf-release serialization.
"""

import os
import numpy as np

B, H, S, D = 2, 16, 2048, 64
N_CORES = 8
PAIRS = (B * H) // N_CORES  # 4 (b,h) pairs per core
NQ = S // 512               # 4 q quarters
NJ = 512 // 128             # 4 output row-blocks per quarter
SCALE = 1.0 / 32.0          # 1/sqrt(HIDDEN_SIZE=1024)

PV_LAG = int(os.environ.get("PV_LAG", "4"))   # in exp-groups
E_BUFS = int(os.environ.get("E_BUFS", "6"))
GROUP = int(os.environ.get("GROUP", "2"))     # k-tiles per exp op
SBUFS = int(os.environ.get("SBUFS", "3"))     # scores PSUM buffers
SCHN = int(os.environ.get("SCHN", "25"))      # k-subtiles handled by DVE sch
SCH_ROUND = os.environ.get("SCH_ROUND", "round")
TAILFREE = int(os.environ.get("TAILFREE", "3"))  # sch-free trailing chunks

# averaged-Schraudolph params (see sch_opt.py): E = v(I1) + W2*v(I1+GAP),
# I1 = round(s*C1 + C2), v(i) = value of bf16 with bits i
C1 = 128 * np.log2(np.e) / 32.0
D1 = -106.91008719227514
GAP = -62
C2 = 16256.0 + D1 + (0.5 if SCH_ROUND == "trunc" else 0.0)

_cached = {}


def _build_nc(kt_tiles, schn):
    import concourse.bacc as bacc_mod
    import concourse.tile as tile
    from concourse import mybir
    from contextlib import ExitStack

    f32 = mybir.dt.float32
    f32r = mybir.dt.float32r
    bf16 = mybir.dt.bfloat16
    fp8 = mybir.dt.float8e4
    i16 = mybir.dt.int16
    Alu = mybir.AluOpType
    Exp = mybir.ActivationFunctionType.Exp
    DR = mybir.MatmulPerfMode.DoubleRow
    sk = kt_tiles * 128

    nc = bacc_mod.Bacc("TRN2")
    # qx: [pair, d(64), slice(2), q]: slice0 = Q8^T, slice1 = R8^T
    qx = nc.dram_tensor("qx", [PAIRS, 64, 2, S], fp8, kind="ExternalInput")
    # kx: [pair, d(64), slice(2), k]: both slices = K8^T
    kx = nc.dram_tensor("kx", [PAIRS, 64, 2, sk], fp8, kind="ExternalInput")
    # vx preswizzled: [pair, p, t, j] = V1[pair, t*128+p, j], V1 = [V | 1]
    vx = nc.dram_tensor("vx", [PAIRS, 128, kt_tiles, D + 1], bf16,
                        kind="ExternalInput")
    idn = nc.dram_tensor("idn", [65, 65], f32, kind="ExternalInput")
    # pair-0 warm bundle: {kx tile0, qx quarter0} -> one early DMA
    hd = nc.dram_tensor("hd", [64, 2, 640], fp8, kind="ExternalInput")
    # out is written permuted as [pair, quarter, p, j, d] with
    # q = quarter*512 + j*128 + p; the host unshuffles.
    out = nc.dram_tensor("out", [PAIRS, NQ, 128, NJ, D], f32,
                         kind="ExternalOutput")

    ctx = ExitStack()
    with tile.TileContext(nc) as tc:
        with ctx:
            consts = ctx.enter_context(tc.tile_pool(name="consts", bufs=1))
            qk_pool = ctx.enter_context(tc.tile_pool(name="qk", bufs=2))
            v_pool = ctx.enter_context(tc.tile_pool(name="v", bufs=2))
            e_pool = ctx.enter_context(tc.tile_pool(name="e", bufs=E_BUFS))
            i1_pool = ctx.enter_context(tc.tile_pool(name="i1", bufs=E_BUFS))
            i2_pool = ctx.enter_context(tc.tile_pool(name="i2", bufs=E_BUFS))
            es_pool = ctx.enter_context(tc.tile_pool(name="es", bufs=E_BUFS))
            ot_pool = ctx.enter_context(tc.tile_pool(name="ot", bufs=2))
            o_pool = ctx.enter_context(tc.tile_pool(name="o", bufs=4))
            r_pool = ctx.enter_context(tc.tile_pool(name="r", bufs=4))
            ps_s = ctx.enter_context(
                tc.tile_pool(name="ps_s", bufs=SBUFS, space="PSUM"))
            ps_a = ctx.enter_context(
                tc.tile_pool(name="ps_a", bufs=1, space="PSUM"))
            ps_t = ctx.enter_context(
                tc.tile_pool(name="ps_t", bufs=1, space="PSUM"))

            id_sb = consts.tile([65, 65], f32, tag="ident")
            head_sb = consts.tile([64, 2, 640], fp8, tag="head")

            # Two-stage epilogue: stage 1 (the PSUM->SBUF copy) is issued
            # as soon as the accumulation finishes; stage 2 (PE transposes +
            # reciprocal + normalize + out-DMA) is deferred past the next
            # chunk's QK matmuls so the transposes never head-of-line block
            # QK on the PE queue while waiting for the DVE copy.
            epi2_q = []
            pv_q = []

            def epi_stage1(p, qq, acc):
                ot_sb = ot_pool.tile([65, 512], f32, tag="ot")
                nc.vector.tensor_copy(ot_sb, acc)
                epi2_q.append((p, qq, ot_sb))

            def epi_stage2(final=False):
                if not epi2_q:
                    return
                p, qq, ot_sb = epi2_q.pop(0)
                if final:
                    pt = ps_s.tile([128, NJ, 65], f32, tag="scores")
                else:
                    pt = ps_t.tile([128, NJ, 65], f32, tag="pt")
                for j in range(NJ):
                    nc.tensor.transpose(
                        pt[:, j, :], ot_sb[:, j * 128:(j + 1) * 128],
                        id_sb)
                ptf = pt
                r_sb = r_pool.tile([128, NJ], f32, tag="r")
                nc.vector.reciprocal(r_sb, ptf[:, :, 64])
                o_sb = o_pool.tile([128, NJ, D], f32, tag="o")
                rb = r_sb[:, :].unsqueeze(2).broadcast_to([128, NJ, D])
                nc.vector.scalar_tensor_tensor(
                    o_sb, ptf[:, :, 0:D], 1.0, rb, Alu.mult, Alu.mult)
                if final:
                    nc.sync.dma_start(out[p, qq], o_sb)
                else:
                    nc.gpsimd.dma_start(out[p, qq], o_sb)

            def pop_pv():
                acc_, v_, t_, e_, i_, tag_ = pv_q.pop(0)
                nc.tensor.matmul(
                    acc_[:, :], lhsT=v_[:, t_, :], rhs=e_[:, i_, :],
                    start=(t_ == 0), stop=(t_ == kt_tiles - 1))
                if t_ == kt_tiles - 1:
                    epi_stage1(*tag_, acc_)

            pair_tiles = {}

            def load_pair(p):
                if p in pair_tiles or p >= PAIRS:
                    return
                qx_sb = qk_pool.tile([64, 2, S], fp8, tag="qx")
                kx_sb = qk_pool.tile([64, 2, sk], fp8, tag="kx")
                vx_sb = v_pool.tile([128, kt_tiles, D + 1], bf16, tag="vx")
                if p == 0:
                    nc.sync.dma_start(head_sb, hd[:])
                    # bulk loads exclude the head regions; ordered by use
                    if sk > 128:
                        nc.sync.dma_start(kx_sb[:, :, 128:], kx[p][:, :, 128:])
                    nc.sync.dma_start(vx_sb, vx[p])
                    nc.sync.dma_start(qx_sb[:, :, 512:], qx[p][:, :, 512:])
                    nc.sync.dma_start(id_sb, idn[:])
                else:
                    nc.sync.dma_start(kx_sb, kx[p])
                    nc.sync.dma_start(qx_sb, qx[p])
                    nc.sync.dma_start(vx_sb, vx[p])
                pair_tiles[p] = (qx_sb, kx_sb, vx_sb)

            subtiles = [(p, qq, t) for p in range(PAIRS)
                        for qq in range(NQ) for t in range(kt_tiles)]
            warm = min(int(os.environ.get("WARM", "1")), len(subtiles))
            chunks = [subtiles[:warm]] if warm else []
            i = warm
            while i < len(subtiles):
                chunks.append(subtiles[i:i + GROUP])
                i += GROUP
            n_final = (kt_tiles + GROUP - 1) // GROUP
            n_chunks = len(chunks)

            accs = {}
            qk_ps = {}

            def emit_qk(ci):
                if ci >= n_chunks:
                    return
                chunk = chunks[ci]
                for (p, qq, t) in chunk:
                    load_pair(p)
                    load_pair(p + 1)
                ng = len(chunk)
                ps = ps_s.tile([128, ng, 512], f32, tag="scores")
                for i_, (p, qq, t) in enumerate(chunk):
                    qx_sb, kx_sb, _ = pair_tiles[p]
                    if p == 0 and t == 0:
                        lhsT = head_sb[:, :, 0:128]
                    else:
                        lhsT = kx_sb[:, :, t * 128:(t + 1) * 128]
                    if p == 0 and qq == 0:
                        rhs = head_sb[:, :, 128:640]
                    else:
                        rhs = qx_sb[:, :, qq * 512:(qq + 1) * 512]
                    nc.tensor.matmul(ps[:, i_, :], lhsT=lhsT, rhs=rhs,
                                     start=True, stop=True, perf_mode=DR)
                qk_ps[ci] = ps

            emit_qk(0)
            for ci, chunk in enumerate(chunks):
                for (p, qq, t) in chunk:
                    if (p, qq) not in accs:
                        accs[(p, qq)] = ps_a.tile(
                            [65, 512], f32, tag="acc", name=f"acc_{p}_{qq}")
                ng = len(chunk)
                ps = qk_ps.pop(ci)
                e_sb = e_pool.tile([128, GROUP, 512], bf16, tag="e")
                # per-chunk sch quota: last `ns` subtiles go to DVE; the
                # trailing chunks stay on ACT so the drain isn't delayed by
                # the slower DVE path
                eff = max(n_chunks - TAILFREE, 1)
                if ci < eff:
                    ns = (ci * schn) // eff - ((ci - 1) * schn) // eff
                else:
                    ns = 0
                ns = min(ns, ng)
                na = ng - ns
                es_sb = None
                # sch op1 goes FIRST on the DVE queue: it reads this chunk's
                # scores psum, so running it early releases the buffer for
                # the QK two chunks ahead
                if ns:
                    i1_sb = i1_pool.tile([128, ns, 512], bf16, tag="i1")
                    i2_sb = i2_pool.tile([128, ns, 512], bf16, tag="i2")
                    es_sb = es_pool.tile([128, ns, 512], bf16, tag="es")
                    nc.vector.tensor_scalar(
                        i1_sb.bitcast(i16), ps[:, na:ng, :],
                        C1, C2, Alu.mult, Alu.add)
                # QK for the NEXT chunk goes early on the PE queue so the
                # exp engines never wait behind this chunk's PV batch.
                emit_qk(ci + 1)
                if na:
                    nc.scalar.activation(e_sb[:, :na, :], ps[:, :na, :],
                                         Exp, scale=SCALE)
                if ns:
                    nc.vector.tensor_scalar(
                        i2_sb.bitcast(i16), i1_sb.bitcast(i16),
                        GAP, None, Alu.add)
                    nc.vector.tensor_tensor(
                        es_sb, i1_sb, i2_sb, Alu.add)
                epi_stage2()
                lag = (GROUP * 1 if ci >= len(chunks) - n_final
                       else GROUP * PV_LAG)
                while len(pv_q) > lag:
                    pop_pv()
                for i_, (p, qq, t) in enumerate(chunk):
                    if i_ < na:
                        pv_q.append((accs[(p, qq)], pair_tiles[p][2], t,
                                     e_sb, i_, (p, qq)))
                    else:
                        pv_q.append((accs[(p, qq)], pair_tiles[p][2], t,
                                     es_sb, i_ - na, (p, qq)))
            while pv_q:
                pop_pv()
            while len(epi2_q) > 1:
                epi_stage2()
            epi_stage2(final=True)

    nc.finalize()
    return nc


def _get_nc(kt_tiles, schn=SCHN):
    key = ("nc", kt_tiles, schn)
    if key not in _cached:
        _cached[key] = _build_nc(kt_tiles, schn)
    return _cached[key]


def _make_in_maps(query, key, value, mask, kt_tiles, kept):
    import ml_dtypes
    fp8 = ml_dtypes.float8_e4m3
    bf16 = ml_dtypes.bfloat16
    sk = kt_tiles * 128
    in_maps = []
    ident = np.eye(65, dtype=np.float32)
    for ci in range(N_CORES):
        h0 = (ci * PAIRS) % H
        b = (ci * PAIRS) // H
        idx = kept[b]
        nk = idx.shape[0]
        qs = query[b, h0:h0 + PAIRS]          # [PAIRS, S, D]
        ks = key[b, h0:h0 + PAIRS][:, idx]    # [PAIRS, nk, D] compacted
        vs = value[b, h0:h0 + PAIRS][:, idx]

        q8 = qs.astype(fp8)
        r8 = (qs - q8.astype(np.float32)).astype(fp8)
        qxa = np.empty((PAIRS, D, 2, S), dtype=fp8)
        qxa[:, :, 0, :] = q8.transpose(0, 2, 1)
        qxa[:, :, 1, :] = r8.transpose(0, 2, 1)

        k8f = np.zeros((PAIRS, D, sk), dtype=np.float32)
        k8f[:, :, :nk] = ks.transpose(0, 2, 1)
        k8 = k8f.astype(fp8)
        kxa = np.empty((PAIRS, D, 2, sk), dtype=fp8)
        kxa[:, :, 0, :] = k8
        kxa[:, :, 1, :] = k8

        v1 = np.zeros((PAIRS, sk, D + 1), dtype=np.float32)
        v1[:, :nk, :D] = vs
        v1[:, :nk, D] = 1.0
        vxa = np.ascontiguousarray(
            v1.reshape(PAIRS, kt_tiles, 128, D + 1).transpose(0, 2, 1, 3)
        ).astype(bf16)

        hda = np.empty((D, 2, 640), dtype=fp8)
        hda[:, :, 0:128] = kxa[0, :, :, 0:128]
        hda[:, :, 128:640] = qxa[0, :, :, 0:512]
        in_maps.append({"qx": qxa, "kx": kxa, "vx": vxa, "idn": ident,
                        "hd": hda})
    return in_maps


def kernel(query, key, value, mask, _trace=False):
    import sys
    for pth in ("/opt/trn_rl_repo", "/opt/pypackages"):
        if pth not in sys.path and os.path.isdir(pth):
            sys.path.append(pth)
    from concourse.bass_utils import run_bass_kernel_spmd

    query = np.asarray(query)
    key = np.asarray(key)
    value = np.asarray(value)
    mask = np.asarray(mask)

    kept = [np.nonzero(mask[b] != 0)[0] for b in range(B)]
    max_k = max(max(idx.shape[0] for idx in kept), 1)
    kt_tiles = (max_k + 127) // 128
    nc = _get_nc(kt_tiles)
    in_maps = _make_in_maps(query, key, value, mask, kt_tiles, kept)
    res = run_bass_kernel_spmd(
        nc, in_maps, core_ids=list(range(N_CORES)), trace=_trace)
    _cached["last_result"] = res
    full = np.empty((B, H, S, D), dtype=np.float32)
    for ci in range(N_CORES):
        h0 = (ci * PAIRS) % H
        b = (ci * PAIRS) // H
        o = res.results[ci]["out"]  # [PAIRS, NQ, 128, NJ, D]
        full[b, h0:h0 + PAIRS] = o.transpose(0, 1, 3, 2, 4).reshape(
            PAIRS, S, D)
    return full


# revision 5
# speedup vs baseline: 1.0424x; 1.0424x over previous
"""Masked dot-product attention on 8 Trainium2 NeuronCores — v2.

Problem: B=2, H=16, S=2048, D=64 fp32; scores = QK^T/sqrt(1024),
key-mask [B,S] with -1e9 on masked keys, softmax over keys, out = W @ V.

v2 strategy (data-parallel over the 32 (b,h) pairs, 4 per core):
 - K/V host-compacted to kept keys (masked keys get exactly-zero weight),
   zero-padded to a multiple of 128. Pad rows have all-zero V|ones rows so
   they are annihilated by the PV matmul.
 - QK runs as ONE fp8 DoubleRow matmul per (k-tile, q-quarter) with
   contraction 128 = [Q8 | R8] x [K8 | K8]: slice 0 carries e4m3(Q), slice 1
   the Q-quantization residual e4m3(Q - Q8) against the same K8. Cost is
   0.5 cycles/row (2x over f32r); Q-side quantization error cancels
   in-matmul, only K8's ~3.6% per-element quantization remains (output rel
   err ~9e-3 vs the 2e-2 gate).
 - exp is split between engines at k-subtile granularity: ACT computes
   exact bf16 exp on each group's first subtiles; DVE approximates the
   group's last SCHN-pattern subtiles with a ONE-OP scale-normalized
   Schraudolph: E = bf16-bits of i16(round(s*C1 + C2S)). Its +/-4%
   sawtooth averages out across each 128-key block inside every softmax
   row (output rel err ~1.3e-2 vs the 2e-2 gate), and one DVE op per
   subtile lets DVE absorb ~24% of the exp work.
 - PV: lhsT = V1 = [V | ones] bf16 [128k, 65]; PSUM acc [65, 512]
   accumulates numerator and denominator together.
 - Epilogue per (pair, quarter): DVE copies acc -> SBUF; 4 PE transposes
   (f32) bring q onto partitions as pt [128, 4, 65]; one batched
   DVE reciprocal [128, 4]; one DVE scalar_tensor_tensor multiplies all
   4x64 output columns by the per-(partition, j) reciprocal broadcast along
   d (free-dim 0-stride AP).
 - PSUM: scores [128, GROUP=2, 512] x3 bufs (6 banks) + acc (1) + pt (1)
   = 8 banks; 3 score buffers break the QK->exp->buf-release serialization.
"""

import os
import numpy as np

B, H, S, D = 2, 16, 2048, 64
N_CORES = 8
PAIRS = (B * H) // N_CORES  # 4 (b,h) pairs per core
NQ = S // 512               # 4 q quarters
NJ = 512 // 128             # 4 output row-blocks per quarter
SCALE = 1.0 / 32.0          # 1/sqrt(HIDDEN_SIZE=1024)

PV_LAG = int(os.environ.get("PV_LAG", "3"))   # in exp-groups
E_BUFS = int(os.environ.get("E_BUFS", "6"))
GROUP = int(os.environ.get("GROUP", "2"))     # k-tiles per exp op
SBUFS = int(os.environ.get("SBUFS", "3"))     # scores PSUM buffers
SCHN = int(os.environ.get("SCHN", "31"))      # k-subtiles handled by DVE sch
SCH_ROUND = os.environ.get("SCH_ROUND", "round")
TAILFREE = int(os.environ.get("TAILFREE", "3"))  # sch-free trailing chunks

# averaged-Schraudolph params (see sch_opt.py): E = v(I1) + W2*v(I1+GAP),
# I1 = round(s*C1 + C2), v(i) = value of bf16 with bits i
C1 = 128 * np.log2(np.e) / 32.0
# single-point variant (scale-normalized): E = v(round(s*C1 + C2S));
# max rel err 4% but k-block averaging inside each softmax keeps the
# output at ~1.3e-2 (see precision study)
D1S = -7.0741
C2S = 16256.0 + D1S + (0.5 if SCH_ROUND == "trunc" else 0.0)

_cached = {}


def _build_nc(kt_tiles, schn):
    import concourse.bacc as bacc_mod
    import concourse.tile as tile
    from concourse import mybir
    from contextlib import ExitStack

    f32 = mybir.dt.float32
    f32r = mybir.dt.float32r
    bf16 = mybir.dt.bfloat16
    fp8 = mybir.dt.float8e4
    i16 = mybir.dt.int16
    Alu = mybir.AluOpType
    Exp = mybir.ActivationFunctionType.Exp
    DR = mybir.MatmulPerfMode.DoubleRow
    sk = kt_tiles * 128

    nc = bacc_mod.Bacc("TRN2")
    # qx: [pair, d(64), slice(2), q]: slice0 = Q8^T, slice1 = R8^T
    qx = nc.dram_tensor("qx", [PAIRS, 64, 2, S], fp8, kind="ExternalInput")
    # kx: [pair, d(64), slice(2), k]: both slices = K8^T
    kx = nc.dram_tensor("kx", [PAIRS, 64, 2, sk], fp8, kind="ExternalInput")
    # vx preswizzled: [pair, p, t, j] = V1[pair, t*128+p, j], V1 = [V | 1]
    vx = nc.dram_tensor("vx", [PAIRS, 128, kt_tiles, D + 1], bf16,
                        kind="ExternalInput")
    idn = nc.dram_tensor("idn", [65, 65], f32, kind="ExternalInput")
    # pair-0 warm bundle: {kx tile0, qx quarter0} -> one early DMA
    hd = nc.dram_tensor("hd", [64, 2, 640], fp8, kind="ExternalInput")
    # out is written permuted as [pair, quarter, p, j, d] with
    # q = quarter*512 + j*128 + p; the host unshuffles.
    out = nc.dram_tensor("out", [PAIRS, NQ, 128, NJ, D], f32,
                         kind="ExternalOutput")

    ctx = ExitStack()
    with tile.TileContext(nc) as tc:
        with ctx:
            consts = ctx.enter_context(tc.tile_pool(name="consts", bufs=1))
            qk_pool = ctx.enter_context(tc.tile_pool(name="qk", bufs=2))
            v_pool = ctx.enter_context(tc.tile_pool(name="v", bufs=2))
            e_pool = ctx.enter_context(tc.tile_pool(name="e", bufs=E_BUFS))
            i1_pool = ctx.enter_context(tc.tile_pool(name="i1", bufs=E_BUFS))
            i2_pool = ctx.enter_context(tc.tile_pool(name="i2", bufs=E_BUFS))
            es_pool = ctx.enter_context(tc.tile_pool(name="es", bufs=E_BUFS))
            ot_pool = ctx.enter_context(tc.tile_pool(name="ot", bufs=2))
            o_pool = ctx.enter_context(tc.tile_pool(name="o", bufs=4))
            r_pool = ctx.enter_context(tc.tile_pool(name="r", bufs=4))
            ps_s = ctx.enter_context(
                tc.tile_pool(name="ps_s", bufs=SBUFS, space="PSUM"))
            ps_a = ctx.enter_context(
                tc.tile_pool(name="ps_a", bufs=1, space="PSUM"))
            ps_t = ctx.enter_context(
                tc.tile_pool(name="ps_t", bufs=1, space="PSUM"))

            id_sb = consts.tile([65, 65], f32, tag="ident")
            head_sb = consts.tile([64, 2, 640], fp8, tag="head")

            # Two-stage epilogue: stage 1 (the PSUM->SBUF copy) is issued
            # as soon as the accumulation finishes; stage 2 (PE transposes +
            # reciprocal + normalize + out-DMA) is deferred past the next
            # chunk's QK matmuls so the transposes never head-of-line block
            # QK on the PE queue while waiting for the DVE copy.
            epi2_q = []
            pv_q = []

            def epi_stage1(p, qq, acc):
                ot_sb = ot_pool.tile([65, 512], f32, tag="ot")
                nc.vector.tensor_copy(ot_sb, acc)
                epi2_q.append((p, qq, ot_sb))

            def epi_stage2(final=False):
                if not epi2_q:
                    return
                p, qq, ot_sb = epi2_q.pop(0)
                if final:
                    pt = ps_s.tile([128, NJ, 65], f32, tag="scores")
                else:
                    pt = ps_t.tile([128, NJ, 65], f32, tag="pt")
                for j in range(NJ):
                    nc.tensor.transpose(
                        pt[:, j, :], ot_sb[:, j * 128:(j + 1) * 128],
                        id_sb)
                ptf = pt
                r_sb = r_pool.tile([128, NJ], f32, tag="r")
                nc.vector.reciprocal(r_sb, ptf[:, :, 64])
                o_sb = o_pool.tile([128, NJ, D], f32, tag="o")
                rb = r_sb[:, :].unsqueeze(2).broadcast_to([128, NJ, D])
                nc.vector.scalar_tensor_tensor(
                    o_sb, ptf[:, :, 0:D], 1.0, rb, Alu.mult, Alu.mult)
                if final:
                    nc.sync.dma_start(out[p, qq], o_sb)
                else:
                    nc.gpsimd.dma_start(out[p, qq], o_sb)

            def pop_pv():
                acc_, v_, t_, e_, i_, tag_ = pv_q.pop(0)
                nc.tensor.matmul(
                    acc_[:, :], lhsT=v_[:, t_, :], rhs=e_[:, i_, :],
                    start=(t_ == 0), stop=(t_ == kt_tiles - 1))
                if t_ == kt_tiles - 1:
                    epi_stage1(*tag_, acc_)

            pair_tiles = {}

            def load_pair(p):
                if p in pair_tiles or p >= PAIRS:
                    return
                qx_sb = qk_pool.tile([64, 2, S], fp8, tag="qx")
                kx_sb = qk_pool.tile([64, 2, sk], fp8, tag="kx")
                vx_sb = v_pool.tile([128, kt_tiles, D + 1], bf16, tag="vx")
                if p == 0:
                    nc.sync.dma_start(head_sb, hd[:])
                    # bulk loads exclude the head regions; ordered by use
                    if sk > 128:
                        nc.sync.dma_start(kx_sb[:, :, 128:], kx[p][:, :, 128:])
                    nc.sync.dma_start(vx_sb, vx[p])
                    nc.sync.dma_start(qx_sb[:, :, 512:], qx[p][:, :, 512:])
                    nc.sync.dma_start(id_sb, idn[:])
                else:
                    nc.sync.dma_start(kx_sb, kx[p])
                    nc.sync.dma_start(qx_sb, qx[p])
                    nc.sync.dma_start(vx_sb, vx[p])
                pair_tiles[p] = (qx_sb, kx_sb, vx_sb)

            subtiles = [(p, qq, t) for p in range(PAIRS)
                        for qq in range(NQ) for t in range(kt_tiles)]
            warm = min(int(os.environ.get("WARM", "1")), len(subtiles))
            chunks = [subtiles[:warm]] if warm else []
            i = warm
            while i < len(subtiles):
                chunks.append(subtiles[i:i + GROUP])
                i += GROUP
            n_final = (kt_tiles + GROUP - 1) // GROUP
            n_chunks = len(chunks)

            accs = {}
            qk_ps = {}

            def emit_qk(ci):
                if ci >= n_chunks:
                    return
                chunk = chunks[ci]
                for (p, qq, t) in chunk:
                    load_pair(p)
                    load_pair(p + 1)
                ng = len(chunk)
                ps = ps_s.tile([128, ng, 512], f32, tag="scores")
                for i_, (p, qq, t) in enumerate(chunk):
                    qx_sb, kx_sb, _ = pair_tiles[p]
                    if p == 0 and t == 0:
                        lhsT = head_sb[:, :, 0:128]
                    else:
                        lhsT = kx_sb[:, :, t * 128:(t + 1) * 128]
                    if p == 0 and qq == 0:
                        rhs = head_sb[:, :, 128:640]
                    else:
                        rhs = qx_sb[:, :, qq * 512:(qq + 1) * 512]
                    nc.tensor.matmul(ps[:, i_, :], lhsT=lhsT, rhs=rhs,
                                     start=True, stop=True, perf_mode=DR)
                qk_ps[ci] = ps

            emit_qk(0)
            for ci, chunk in enumerate(chunks):
                for (p, qq, t) in chunk:
                    if (p, qq) not in accs:
                        accs[(p, qq)] = ps_a.tile(
                            [65, 512], f32, tag="acc", name=f"acc_{p}_{qq}")
                ng = len(chunk)
                ps = qk_ps.pop(ci)
                e_sb = e_pool.tile([128, GROUP, 512], bf16, tag="e")
                # per-chunk sch quota: last `ns` subtiles go to DVE; the
                # trailing chunks stay on ACT so the drain isn't delayed by
                # the slower DVE path
                eff = max(n_chunks - TAILFREE, 1)
                if 0 < ci < eff:
                    ns = (ci * schn) // eff - ((ci - 1) * schn) // eff
                else:
                    ns = 0
                ns = min(ns, ng)
                na = ng - ns
                # sch op goes FIRST on the DVE queue: it reads this chunk's
                # scores psum, so running it early releases the buffer for
                # the QK two chunks ahead. Single-point Schraudolph: the i16
                # result IS the bf16 E value - one DVE op per sch subtile.
                i1_sb = None
                if ns:
                    i1_sb = i1_pool.tile([128, ns, 512], bf16, tag="i1")
                    nc.vector.tensor_scalar(
                        i1_sb.bitcast(i16), ps[:, na:ng, :],
                        C1, C2S, Alu.mult, Alu.add)
                # QK for the NEXT chunk goes early on the PE queue so the
                # exp engines never wait behind this chunk's PV batch; the
                # PV drain (whose quarter-completing pops emit the stage-1
                # copies on DVE) ALSO goes before this chunk's op2/merge so
                # the single-buffered acc slot is released as early as
                # possible.
                emit_qk(ci + 1)
                lag = (GROUP * 1 if ci >= len(chunks) - n_final
                       else GROUP * PV_LAG)
                while len(pv_q) > lag:
                    pop_pv()
                if na:
                    nc.scalar.activation(e_sb[:, :na, :], ps[:, :na, :],
                                         Exp, scale=SCALE)
                epi_stage2()
                for i_, (p, qq, t) in enumerate(chunk):
                    if i_ < na:
                        pv_q.append((accs[(p, qq)], pair_tiles[p][2], t,
                                     e_sb, i_, (p, qq)))
                    else:
                        pv_q.append((accs[(p, qq)], pair_tiles[p][2], t,
                                     i1_sb, i_ - na, (p, qq)))
            while pv_q:
                pop_pv()
            while len(epi2_q) > 1:
                epi_stage2()
            epi_stage2(final=True)

    nc.finalize()
    return nc


def _get_nc(kt_tiles, schn=SCHN):
    key = ("nc", kt_tiles, schn)
    if key not in _cached:
        _cached[key] = _build_nc(kt_tiles, schn)
    return _cached[key]


def _make_in_maps(query, key, value, mask, kt_tiles, kept):
    import ml_dtypes
    fp8 = ml_dtypes.float8_e4m3
    bf16 = ml_dtypes.bfloat16
    sk = kt_tiles * 128
    in_maps = []
    ident = np.eye(65, dtype=np.float32)
    for ci in range(N_CORES):
        h0 = (ci * PAIRS) % H
        b = (ci * PAIRS) // H
        idx = kept[b]
        nk = idx.shape[0]
        qs = query[b, h0:h0 + PAIRS]          # [PAIRS, S, D]
        ks = key[b, h0:h0 + PAIRS][:, idx]    # [PAIRS, nk, D] compacted
        vs = value[b, h0:h0 + PAIRS][:, idx]

        q8 = qs.astype(fp8)
        r8 = (qs - q8.astype(np.float32)).astype(fp8)
        qxa = np.empty((PAIRS, D, 2, S), dtype=fp8)
        qxa[:, :, 0, :] = q8.transpose(0, 2, 1)
        qxa[:, :, 1, :] = r8.transpose(0, 2, 1)

        k8f = np.zeros((PAIRS, D, sk), dtype=np.float32)
        k8f[:, :, :nk] = ks.transpose(0, 2, 1)
        k8 = k8f.astype(fp8)
        kxa = np.empty((PAIRS, D, 2, sk), dtype=fp8)
        kxa[:, :, 0, :] = k8
        kxa[:, :, 1, :] = k8

        v1 = np.zeros((PAIRS, sk, D + 1), dtype=np.float32)
        v1[:, :nk, :D] = vs
        v1[:, :nk, D] = 1.0
        vxa = np.ascontiguousarray(
            v1.reshape(PAIRS, kt_tiles, 128, D + 1).transpose(0, 2, 1, 3)
        ).astype(bf16)

        hda = np.empty((D, 2, 640), dtype=fp8)
        hda[:, :, 0:128] = kxa[0, :, :, 0:128]
        hda[:, :, 128:640] = qxa[0, :, :, 0:512]
        in_maps.append({"qx": qxa, "kx": kxa, "vx": vxa, "idn": ident,
                        "hd": hda})
    return in_maps


def kernel(query, key, value, mask, _trace=False):
    import sys
    for pth in ("/opt/trn_rl_repo", "/opt/pypackages"):
        if pth not in sys.path and os.path.isdir(pth):
            sys.path.append(pth)
    from concourse.bass_utils import run_bass_kernel_spmd

    query = np.asarray(query)
    key = np.asarray(key)
    value = np.asarray(value)
    mask = np.asarray(mask)

    kept = [np.nonzero(mask[b] != 0)[0] for b in range(B)]
    max_k = max(max(idx.shape[0] for idx in kept), 1)
    kt_tiles = (max_k + 127) // 128
    nc = _get_nc(kt_tiles)
    in_maps = _make_in_maps(query, key, value, mask, kt_tiles, kept)
    res = run_bass_kernel_spmd(
        nc, in_maps, core_ids=list(range(N_CORES)), trace=_trace)
    _cached["last_result"] = res
    full = np.empty((B, H, S, D), dtype=np.float32)
    for ci in range(N_CORES):
        h0 = (ci * PAIRS) % H
        b = (ci * PAIRS) // H
        o = res.results[ci]["out"]  # [PAIRS, NQ, 128, NJ, D]
        full[b, h0:h0 + PAIRS] = o.transpose(0, 1, 3, 2, 4).reshape(
            PAIRS, S, D)
    return full
